# revision 7
# baseline (speedup 1.0000x reference)
"""Trainium2 Bass kernel for GCN ExitBlock: out = (adj @ (x @ gc_W) + gc_b) @ fc_W + fc_b.

Strategy (8 NeuronCores, SPMD, no collectives):
  - Reassociate: out = ((adj @ x) @ gc_W + gc_b) @ fc_W + fc_b.  The big
    streaming matmul g = adj @ x then uses x in its NATURAL [k, 32] layout as
    the PE's stationary operand -- no transposes and no per-tile prep work.
  - Row-shard the output: core c computes rows [1500c, 1500(c+1)).
  - Host pre-transposes adj: core c receives adjT_c = adj[rows_c, :].T
    ([12032, 1500] zero-padded, contiguous) so the contraction dim lands on
    SBUF partitions.  k-tiles are batched into multi-tile slabs (p-interleaved:
    slab row p holds k = k0 + G*p + j) keeping per-partition DMA contiguous at
    G*6000 B; x is pre-permuted on the host to match.
  - Per sub-tile: gT[32,1500] += x_tile.T @ adjT_slab in f32r (1-pass matmuls,
    tf32-class precision, fp32 PSUM accumulate).
  - Epilogue: hT = gc_W.T @ gT; outT = fc_W.T @ hT + (fc_W.T gc_b + fc_b);
    biases folded into a single [16,1] vector via a tiny matmul.
  - Host gathers the 8 outT blocks ([16, 1500]) and transposes to [12000, 16].

HBM-bound: 72 MB of adj per core @ ~358 GB/s => ~202 us roofline.
"""
import sys

sys.path.insert(0, "/opt/trn_rl_repo")

import numpy as np

N, NHID, NCLASS, NCORES = 12000, 32, 16, 8
R = N // NCORES            # 1500 rows per core
KP = 128                   # partitions per sub-tile
NT = 94                    # sub-tiles (12032 padded k rows)
NPAD = NT * KP             # 12032
GROUPS = [4] * 23 + [2]    # sub-tiles per DMA slab (3 MB / 1.5 MB transfers)
assert sum(GROUPS) == NT
R_SPLITS = [(0, 512), (512, 512), (1024, R - 1024)]           # matmul N<=512

_cached = {}


def _build_nc():
    import concourse.bacc as bacc
    import concourse.mybir as mybir
    from concourse import tile

    f32 = mybir.dt.float32
    f32r = mybir.dt.float32r

    nc = bacc.Bacc()
    xP_d = nc.declare_dram_parameter("xP", [KP, NT * NHID], f32r, isOutput=False)
    adjT_d = nc.declare_dram_parameter("adjT", [NPAD, R], f32r, isOutput=False)
    gcW_d = nc.declare_dram_parameter("gcW", [NHID, NHID], f32r, isOutput=False)
    fcW_d = nc.declare_dram_parameter("fcW", [NHID, NCLASS], f32r, isOutput=False)
    # cvec = [gc_b; 1] followed by [fc_W; fc_b] rows -> bias vector via matmul
    fcWb_d = nc.declare_dram_parameter("fcWb", [NHID + 1, NCLASS], f32, isOutput=False)
    gcb1_d = nc.declare_dram_parameter("gcb1", [NHID + 1, 1], f32, isOutput=False)
    outT_d = nc.declare_dram_parameter("outT", [NCLASS, R], f32, isOutput=True)

    with tile.TileContext(nc) as tc:
        with (
            tc.tile_pool(name="cst", bufs=1) as cst,
            tc.tile_pool(name="adj", bufs=4) as adjp,
            tc.tile_pool(name="ps_g", bufs=1, space="PSUM") as ps_g,
            tc.tile_pool(name="ps_e", bufs=1, space="PSUM") as ps_e,
        ):
            # ---- constant preloads ----
            x_sb = cst.tile([KP, NT, NHID], f32r)
            nc.gpsimd.dma_start(x_sb[:], xP_d.rearrange("p (t j) -> p t j", j=NHID))
            gcW_sb = cst.tile([NHID, NHID], f32r)
            nc.gpsimd.dma_start(gcW_sb[:], gcW_d[:])
            fcW_sb = cst.tile([NHID, NCLASS], f32r)
            nc.gpsimd.dma_start(fcW_sb[:], fcW_d[:])
            fcWb_sb = cst.tile([NHID + 1, NCLASS], f32)
            nc.gpsimd.dma_start(fcWb_sb[:], fcWb_d[:])
            gcb1_sb = cst.tile([NHID + 1, 1], f32)
            nc.gpsimd.dma_start(gcb1_sb[:], gcb1_d[:])

            gps = [ps_g.tile([NHID, n], f32, name=f"gps{j}", tag=f"gps{j}")
                   for j, (_, n) in enumerate(R_SPLITS)]

            # bias vector c = fcWb.T @ [gc_b; 1] = fc_W.T gc_b + fc_b  [16, 1]
            c_ps = ps_e.tile([NCLASS, 1], f32, name="c_ps", tag="c_ps")
            nc.tensor.matmul(c_ps[:], fcWb_sb[:], gcb1_sb[:], start=True, stop=True)
            c_sb = cst.tile([NCLASS, 1], f32)
            nc.vector.tensor_copy(c_sb[:], c_ps[:])

            # ---- main streaming loop: gT += x_tile.T @ adjT_slab ----
            s = 0          # global sub-tile index
            k0 = 0
            ngroups = len(GROUPS)
            for g, G in enumerate(GROUPS):
                a_sb = adjp.tile([KP, 4, R], f32r, name="a_sb", tag="a")
                eng = nc.sync if (g % 2 == 0) else nc.scalar
                eng.dma_start(
                    a_sb[:, :G, :],
                    adjT_d[k0:k0 + KP * G, :].rearrange("(p j) r -> p j r", j=G))
                for j in range(G):
                    st = (s == 0)
                    sp = (s == NT - 1)
                    for q, (c0, cn) in enumerate(R_SPLITS):
                        nc.tensor.matmul(gps[q][:, :], x_sb[:, s, :],
                                         a_sb[:, j, c0:c0 + cn],
                                         start=st, stop=sp)
                    s += 1
                k0 += KP * G

            # ---- epilogue: hT = gcW.T @ gT;  outT = fcW.T @ hT + c ----
            g_sb = cst.tile([NHID, R], f32r)
            hT_sb = cst.tile([NHID, R], f32r)
            o_sb = cst.tile([NCLASS, R], f32)
            for q, (c0, cn) in enumerate(R_SPLITS):
                nc.vector.tensor_copy(g_sb[:, c0:c0 + cn], gps[q][:, :])
                h_ps = ps_e.tile([NHID, 512], f32, name="h_ps", tag="h_ps")
                nc.tensor.matmul(h_ps[:, :cn], gcW_sb[:], g_sb[:, c0:c0 + cn],
                                 start=True, stop=True)
                nc.vector.tensor_copy(hT_sb[:, c0:c0 + cn], h_ps[:, :cn])
                o_ps = ps_e.tile([NCLASS, 512], f32, name="o_ps", tag="o_ps")
                nc.tensor.matmul(o_ps[:, :cn], fcW_sb[:], hT_sb[:, c0:c0 + cn],
                                 start=True, stop=True)
                nc.vector.tensor_scalar_add(o_sb[:, c0:c0 + cn], o_ps[:, :cn],
                                            c_sb[:, 0:1])
            nc.sync.dma_start(outT_d[:], o_sb[:])

    nc.finalize()
    return nc


def _get_nc():
    if "nc" not in _cached:
        _cached["nc"] = _build_nc()
    return _cached["nc"]


def _prep_in_maps(x, adj, gc_W, gc_b, fc_W, fc_b):
    f = np.float32
    x = np.asarray(x, dtype=f)
    adj = np.asarray(adj, dtype=f)

    # x permuted to match the slab interleave: xP[p, s*NHID:(s+1)*NHID] is the
    # stationary operand of sub-tile s, whose partition p holds k = k0+G*p+j.
    xpad = np.zeros((NPAD, NHID), dtype=f)
    xpad[:N] = x
    xP = np.empty((KP, NT, NHID), dtype=f)
    s = 0
    k0 = 0
    for G in GROUPS:
        blk = xpad[k0:k0 + KP * G].reshape(KP, G, NHID)
        for j in range(G):
            xP[:, s, :] = blk[:, j, :]
            s += 1
        k0 += KP * G
    xP = np.ascontiguousarray(xP.reshape(KP, NT * NHID))

    # adjT blocks: [12032, 1500] per core (32 zero-padded k rows)
    adjT = np.zeros((NCORES, NPAD, R), dtype=f)
    adjT[:, :N, :] = adj.reshape(NCORES, R, N).transpose(0, 2, 1)

    gcW = np.ascontiguousarray(np.asarray(gc_W, dtype=f))
    fcW = np.ascontiguousarray(np.asarray(fc_W, dtype=f))
    fcWb = np.ascontiguousarray(
        np.concatenate([fcW, np.asarray(fc_b, dtype=f).reshape(1, NCLASS)], axis=0))
    gcb1 = np.ascontiguousarray(
        np.concatenate([np.asarray(gc_b, dtype=f).reshape(NHID, 1),
                        np.ones((1, 1), dtype=f)], axis=0))
    return [{"xP": xP, "adjT": adjT[c], "gcW": gcW, "fcW": fcW,
             "fcWb": fcWb, "gcb1": gcb1} for c in range(NCORES)]


def run_traced(x, adj, gc_W, gc_b, fc_W, fc_b, trace=False, **kw):
    """Run on the 8 NeuronCores; returns (out [N, NCLASS] f32, BassKernelResults)."""
    from concourse.bass_utils import run_bass_kernel_spmd

    nc = _get_nc()
    in_maps = _prep_in_maps(x, adj, gc_W, gc_b, fc_W, fc_b)
    res = run_bass_kernel_spmd(nc, in_maps, list(range(NCORES)), trace=trace, **kw)
    outT = np.concatenate([res.results[c]["outT"] for c in range(NCORES)], axis=1)
    out = np.ascontiguousarray(outT.T).astype(np.float32, copy=False)
    return out, res


def kernel(x, adj, gc_W, gc_b, fc_W, fc_b):
    out, _ = run_traced(x, adj, gc_W, gc_b, fc_W, fc_b, trace=False)
    return out


# revision 8
# speedup vs baseline: 1.1173x; 1.1173x over previous
"""Trainium2 Bass kernel for GCN ExitBlock: out = (adj @ (x @ gc_W) + gc_b) @ fc_W + fc_b.

Strategy (8 NeuronCores, SPMD, no collectives):
  - Reassociate: out = ((adj @ x) @ gc_W + gc_b) @ fc_W + fc_b.  The big
    streaming matmul g = adj @ x then uses x in its NATURAL [k, 32] layout as
    the PE's stationary operand -- no transposes and no per-tile prep work.
  - Row-shard the output: core c computes rows [1500c, 1500(c+1)).
  - Host pre-transposes adj: core c receives adjT_c = adj[rows_c, :].T
    ([12032, 1500] zero-padded, contiguous) so the contraction dim lands on
    SBUF partitions.  k-tiles are batched into multi-tile slabs (p-interleaved:
    slab row p holds k = k0 + G*p + j) keeping per-partition DMA contiguous at
    G*6000 B; x is pre-permuted on the host to match.
  - Per sub-tile: gT[32,1500] += x_tile.T @ adjT_slab in f32r (1-pass matmuls,
    tf32-class precision, fp32 PSUM accumulate).
  - Epilogue: hT = gc_W.T @ gT; outT = fc_W.T @ hT + (fc_W.T gc_b + fc_b);
    biases folded into a single [16,1] vector via a tiny matmul.
  - Host gathers the 8 outT blocks ([16, 1500]) and transposes to [12000, 16].

HBM-bound: 72 MB of adj per core @ ~358 GB/s => ~202 us roofline.
"""
import sys

sys.path.insert(0, "/opt/trn_rl_repo")

import numpy as np

N, NHID, NCLASS, NCORES = 12000, 32, 16, 8
R = N // NCORES            # 1500 rows per core
KP = 128                   # partitions per sub-tile
NT = 94                    # sub-tiles (12032 padded k rows)
NPAD = NT * KP             # 12032
GROUPS = [4] * 23 + [2]    # sub-tiles per DMA slab (3 MB / 1.5 MB transfers)
assert sum(GROUPS) == NT
R_SPLITS = [(0, 512), (512, 512), (1024, R - 1024)]           # matmul N<=512

_cached = {}


def _build_nc():
    import concourse.bacc as bacc
    import concourse.mybir as mybir
    from concourse import tile

    f32 = mybir.dt.float32
    f32r = mybir.dt.float32r

    nc = bacc.Bacc()
    xP_d = nc.declare_dram_parameter("xP", [KP, NT * NHID], f32r, isOutput=False)
    adjT_d = nc.declare_dram_parameter("adjT", [NPAD, R], f32r, isOutput=False)
    gcW_d = nc.declare_dram_parameter("gcW", [NHID, NHID], f32r, isOutput=False)
    fcW_d = nc.declare_dram_parameter("fcW", [NHID, NCLASS], f32r, isOutput=False)
    # cvec = [gc_b; 1] followed by [fc_W; fc_b] rows -> bias vector via matmul
    fcWb_d = nc.declare_dram_parameter("fcWb", [NHID + 1, NCLASS], f32, isOutput=False)
    gcb1_d = nc.declare_dram_parameter("gcb1", [NHID + 1, 1], f32, isOutput=False)
    outT_d = nc.declare_dram_parameter("outT", [NCLASS, R], f32, isOutput=True)

    with tile.TileContext(nc) as tc:
        with (
            tc.tile_pool(name="cst", bufs=1) as cst,
            tc.tile_pool(name="adj", bufs=4) as adjp,
            tc.tile_pool(name="ps_g", bufs=1, space="PSUM") as ps_g,
            tc.tile_pool(name="ps_e", bufs=1, space="PSUM") as ps_e,
        ):
            # ---- constant preloads ----
            x_sb = cst.tile([KP, NT, NHID], f32r)
            nc.scalar.dma_start(x_sb[:], xP_d.rearrange("p (t j) -> p t j", j=NHID))
            gcW_sb = cst.tile([NHID, NHID], f32r)
            nc.scalar.dma_start(gcW_sb[:], gcW_d[:])
            fcW_sb = cst.tile([NHID, NCLASS], f32r)
            nc.scalar.dma_start(fcW_sb[:], fcW_d[:])
            fcWb_sb = cst.tile([NHID + 1, NCLASS], f32)
            nc.scalar.dma_start(fcWb_sb[:], fcWb_d[:])
            gcb1_sb = cst.tile([NHID + 1, 1], f32)
            nc.scalar.dma_start(gcb1_sb[:], gcb1_d[:])

            gps = [ps_g.tile([NHID, n], f32, name=f"gps{j}", tag=f"gps{j}")
                   for j, (_, n) in enumerate(R_SPLITS)]

            # bias vector c = fcWb.T @ [gc_b; 1] = fc_W.T gc_b + fc_b  [16, 1]
            c_ps = ps_e.tile([NCLASS, 1], f32, name="c_ps", tag="c_ps")
            nc.tensor.matmul(c_ps[:], fcWb_sb[:], gcb1_sb[:], start=True, stop=True)
            c_sb = cst.tile([NCLASS, 1], f32)
            nc.vector.tensor_copy(c_sb[:], c_ps[:])

            # ---- main streaming loop: gT += x_tile.T @ adjT_slab ----
            s = 0          # global sub-tile index
            k0 = 0
            ngroups = len(GROUPS)
            for g, G in enumerate(GROUPS):
                a_sb = adjp.tile([KP, 4, R], f32r, name="a_sb", tag="a")
                eng = nc.sync if (g % 2 == 0) else nc.scalar
                eng.dma_start(
                    a_sb[:, :G, :],
                    adjT_d[k0:k0 + KP * G, :].rearrange("(p j) r -> p j r", j=G))
                for j in range(G):
                    st = (s == 0)
                    sp = (s == NT - 1)
                    for q, (c0, cn) in enumerate(R_SPLITS):
                        nc.tensor.matmul(gps[q][:, :], x_sb[:, s, :],
                                         a_sb[:, j, c0:c0 + cn],
                                         start=st, stop=sp)
                    s += 1
                k0 += KP * G

            # ---- epilogue: hT = gcW.T @ gT;  outT = fcW.T @ hT + c ----
            g_sb = cst.tile([NHID, R], f32r)
            hT_sb = cst.tile([NHID, R], f32r)
            o_sb = cst.tile([NCLASS, R], f32)
            for q, (c0, cn) in enumerate(R_SPLITS):
                nc.vector.tensor_copy(g_sb[:, c0:c0 + cn], gps[q][:, :])
                h_ps = ps_e.tile([NHID, 512], f32, name="h_ps", tag="h_ps")
                nc.tensor.matmul(h_ps[:, :cn], gcW_sb[:], g_sb[:, c0:c0 + cn],
                                 start=True, stop=True)
                nc.vector.tensor_copy(hT_sb[:, c0:c0 + cn], h_ps[:, :cn])
                o_ps = ps_e.tile([NCLASS, 512], f32, name="o_ps", tag="o_ps")
                nc.tensor.matmul(o_ps[:, :cn], fcW_sb[:], hT_sb[:, c0:c0 + cn],
                                 start=True, stop=True)
                nc.vector.tensor_scalar_add(o_sb[:, c0:c0 + cn], o_ps[:, :cn],
                                            c_sb[:, 0:1])
            nc.sync.dma_start(outT_d[:], o_sb[:])

    nc.finalize()
    return nc


def _get_nc():
    if "nc" not in _cached:
        _cached["nc"] = _build_nc()
    return _cached["nc"]


def _prep_in_maps(x, adj, gc_W, gc_b, fc_W, fc_b):
    f = np.float32
    x = np.asarray(x, dtype=f)
    adj = np.asarray(adj, dtype=f)

    # x permuted to match the slab interleave: xP[p, s*NHID:(s+1)*NHID] is the
    # stationary operand of sub-tile s, whose partition p holds k = k0+G*p+j.
    xpad = np.zeros((NPAD, NHID), dtype=f)
    xpad[:N] = x
    xP = np.empty((KP, NT, NHID), dtype=f)
    s = 0
    k0 = 0
    for G in GROUPS:
        blk = xpad[k0:k0 + KP * G].reshape(KP, G, NHID)
        for j in range(G):
            xP[:, s, :] = blk[:, j, :]
            s += 1
        k0 += KP * G
    xP = np.ascontiguousarray(xP.reshape(KP, NT * NHID))

    # adjT blocks: [12032, 1500] per core (32 zero-padded k rows)
    adjT = np.zeros((NCORES, NPAD, R), dtype=f)
    adjT[:, :N, :] = adj.reshape(NCORES, R, N).transpose(0, 2, 1)

    gcW = np.ascontiguousarray(np.asarray(gc_W, dtype=f))
    fcW = np.ascontiguousarray(np.asarray(fc_W, dtype=f))
    fcWb = np.ascontiguousarray(
        np.concatenate([fcW, np.asarray(fc_b, dtype=f).reshape(1, NCLASS)], axis=0))
    gcb1 = np.ascontiguousarray(
        np.concatenate([np.asarray(gc_b, dtype=f).reshape(NHID, 1),
                        np.ones((1, 1), dtype=f)], axis=0))
    return [{"xP": xP, "adjT": adjT[c], "gcW": gcW, "fcW": fcW,
             "fcWb": fcWb, "gcb1": gcb1} for c in range(NCORES)]


def run_traced(x, adj, gc_W, gc_b, fc_W, fc_b, trace=False, **kw):
    """Run on the 8 NeuronCores; returns (out [N, NCLASS] f32, BassKernelResults)."""
    from concourse.bass_utils import run_bass_kernel_spmd

    nc = _get_nc()
    in_maps = _prep_in_maps(x, adj, gc_W, gc_b, fc_W, fc_b)
    res = run_bass_kernel_spmd(nc, in_maps, list(range(NCORES)), trace=trace, **kw)
    outT = np.concatenate([res.results[c]["outT"] for c in range(NCORES)], axis=1)
    out = np.ascontiguousarray(outT.T).astype(np.float32, copy=False)
    return out, res


def kernel(x, adj, gc_W, gc_b, fc_W, fc_b):
    out, _ = run_traced(x, adj, gc_W, gc_b, fc_W, fc_b, trace=False)
    return out
